# revision 24
# baseline (speedup 1.0000x reference)
"""Trainium2 kernel for nn_Dense_RBS_density: rho <- U rho U^T over a batch
of 8 density matrices in the Hamming-weight-2 basis of 32 qubits (dim=496).

The 15 RBS gates act on disjoint qubit pairs, so they commute and fold into a
single orthogonal matrix U (built on host from the 15 angles — negligible
work). In a permuted basis U is block-diagonal with four 124x124 blocks
(each itself made of <=4x4 rotations), so per core (one batch element):
    mm1 bank kt: A^T[mt,kt] = rho'[kt,mt]^T @ B^T[kt,kt]   (16 matmuls)
    mm2 bank mt: out'[mt,kt] = A[mt,kt] @ B^T[kt,kt]       (16 matmuls)

Everything is bf16 (inputs, matmuls, PSUM results, stores): the harness
tolerance (2e-2) dwarfs bf16 rounding (~5e-3), bf16 matmuls stream 1 row per
PE cycle at any p-state (f32r pays 2-4x for 124-wide outputs), and bf16
halves DMA bytes.

Schedule: all four mm1 k-sweeps run back-to-back on the PE while DVE chases
them with PSUM->SBUF at-copies; the four mm2 groups follow; each finished
ps2 bank is DMA'd straight from PSUM to HBM (no SBUF staging, no Activation
copies -> no one-time 1.3us ACT table load). Input is loaded as 4 chunks on
4 different engine DMA queues so the whole load fits in one ~500ns slot.
"""

import itertools
import math

import numpy as np

N_QUBITS = 32
LIST_GATES = [(2 * i, 2 * i + 1) for i in range(15)]
DIM = 496  # C(32, 2)
PT = 124  # partition tile size; 4 * 124 = 496
NT = 4  # number of tiles along each axis
N_CORES = 8
N_WARMUP_MM = 1  # dummy matmul that delays the PE past the chunk-0 release
ROW = DIM + PT  # packed input row: 496 rho' columns + 124 block columns


def _gate_pairs():
    """For each gate (a,b), the list of (k, kp) basis-index pairs rotated by
    the gate: k contains a, kp = same state with a replaced by b."""
    pairs = list(itertools.combinations(range(N_QUBITS), 2))
    idx = {p: k for k, p in enumerate(pairs)}
    out = []
    for a, b in LIST_GATES:
        rot = []
        for p, k in idx.items():
            if (a in p) and (b not in p):
                other = p[0] if p[1] == a else p[1]
                kp = idx[tuple(sorted((other, b)))]
                rot.append((k, kp))
        out.append(rot)
    return out


_GATE_PAIRS = _gate_pairs()


def _build_perm():
    """Basis order that block-diagonalizes the folded U: 105 4-blocks (gate
    pair x gate pair), 30 2-blocks (qubit 30/31 partners), 16 fixed states.
    All blocks land inside aligned 124-wide tiles."""
    pairs = list(itertools.combinations(range(N_QUBITS), 2))
    idx = {p: k for k, p in enumerate(pairs)}
    perm = []
    for a in range(15):
        for b in range(a + 1, 15):
            for x in range(2):
                for y in range(2):
                    perm.append(idx[(2 * a + x, 2 * b + y)])
    for q in (30, 31):
        for a in range(15):
            perm.append(idx[tuple(sorted((2 * a, q)))])
            perm.append(idx[tuple(sorted((2 * a + 1, q)))])
    for a in range(15):
        perm.append(idx[(2 * a, 2 * a + 1)])
    perm.append(idx[(30, 31)])
    return np.array(perm)


_PERM = _build_perm()
_INV_PERM = np.argsort(_PERM)


def _build_u(angles: np.ndarray) -> np.ndarray:
    """Fold the 15 commuting RBS gates into one orthogonal DIMxDIM matrix."""
    u = np.eye(DIM, dtype=np.float64)
    for g, rot in enumerate(_GATE_PAIRS):
        c = math.cos(float(angles[g]))
        s = math.sin(float(angles[g]))
        k = np.array([r[0] for r in rot])
        kp = np.array([r[1] for r in rot])
        rk, rkp = u[k].copy(), u[kp].copy()
        u[k] = c * rk + s * rkp
        u[kp] = -s * rk + c * rkp
    return u


_NC_CACHE = {}


def _build_bass():
    import concourse.bass as bass
    import concourse.mybir as mybir
    import concourse.tile as tile
    from concourse.bass import MemorySpace

    mm_dt = mybir.dt.bfloat16

    nc = bass.Bass("TRN2", target_bir_lowering=False, debug=False)
    # 4 chunks of [rho k-tile; U^T k-tile], 124 rows each, bf16.
    inp_d = nc.dram_tensor("inp", [DIM, ROW], mm_dt,
                           kind="ExternalInput").ap()
    out_d = nc.dram_tensor("out", [DIM, DIM], mm_dt,
                           kind="ExternalOutput").ap()

    with tile.TileContext(nc) as tc:
        with (
            tc.tile_pool(name="consts", bufs=1) as consts,
            tc.tile_pool(name="psum", bufs=1, space=MemorySpace.PSUM) as psum,
        ):
            # [124, 4, 620]: per k-tile, 496 rho' columns + this tile's
            # 124x124 diagonal block of B^T.
            inp_sb = consts.tile([PT, NT, ROW], mm_dt, tag="inp")
            at_sb = consts.tile([PT, NT, DIM], mm_dt, tag="at")
            out_sb = consts.tile([PT, NT, DIM], mm_dt, tag="outs")
            warm_sb = consts.tile([PT, 280], mm_dt, tag="warm")

            dma_is, mm_is, cp_is = [], [], []
            # In this simulator a consumer that parks on a DMA semaphore
            # before the DMA's engine-release pays the full ~1.7us DMA init
            # latency; one that checks after release proceeds immediately.
            # So every DMA-sem consumer must ARRIVE LATE: the PE runs one
            # small warmup matmul (gated on a DVE memset sized so the PE
            # reaches the chunk-0 wait just after the load's 500ns engine
            # slot has drained).
            warm_i = nc.vector.memset(warm_sb, 0.0)
            # input chunks: c0/c2/c3 on SP, c1 on Act (both first-needed
            # chunks get first queue slots; Pool gets NO DMA - SWDGE sems
            # resolve ~2.4us late and Pool must be free for copies anyway)
            engs = [nc.sync, nc.scalar, nc.sync, nc.sync]
            for kt in range(NT):
                dma_is.append(engs[kt].dma_start(
                    inp_sb[:, kt, :],
                    inp_d[PT * kt:PT * (kt + 1), :]))
            # Activation's one-time table load (1.4us), paid while the PE
            # does mm1 - Act's first real copy comes much later
            scratch_sb = consts.tile([PT, 1], mybir.dt.float32, tag="scratch")
            cp_is.append(nc.scalar.copy(scratch_sb, warm_sb[:, :1]))

            # 8 PSUM tiles of two 124-slices each. Tile j=2k+m//2 holds mm1
            # slices (k, 2(j%2)) and (k, 2(j%2)+1); mm2 group g reuses tile
            # 2g+k2//2 long after its mm1 readers finished. Each tile gets
            # BOTH its writes before its single half-copy read, so a matmul
            # never stalls on an in-flight PSUM read (tile-level WAR).
            pb = [psum.tile([PT, 2 * PT], mybir.dt.float32, tag=f"pb{j}",
                            name=f"pb{j}") for j in range(2 * NT)]
            for _ in range(N_WARMUP_MM):
                mm_is.append(nc.tensor.matmul(
                    pb[0][:, :PT], warm_sb[:, :PT], warm_sb[:, :PT],
                    start=True, stop=True))

            def mm1_emit(kt, mt):
                mm_is.append(nc.tensor.matmul(
                    pb[2 * kt + mt // 2][:, (mt % 2) * PT:(mt % 2 + 1) * PT],
                    inp_sb[:, kt, mt * PT:(mt + 1) * PT],
                    inp_sb[:, kt, DIM:ROW],
                    start=True, stop=True,
                ))

            def mm2_emit(g, k2):
                # needs region (m-band k2, n-band g) = at tile g, slice k2
                mm_is.append(nc.tensor.matmul(
                    pb[2 * g + k2 // 2][:, (k2 % 2) * PT:(k2 % 2 + 1) * PT],
                    at_sb[:, g, k2 * PT:(k2 + 1) * PT],
                    inp_sb[:, k2, DIM:ROW],
                    start=True, stop=True,
                ))

            def at_half(kt, h, eng):
                sl = slice(h * 2 * PT, (h + 1) * 2 * PT)
                fn = getattr(eng, "tensor_copy", None) or eng.copy
                cp_is.append(fn(at_sb[:, kt, sl], pb[2 * kt + h]))

            def out_half(g, h, eng):
                sl = slice(h * 2 * PT, (h + 1) * 2 * PT)
                fn = getattr(eng, "tensor_copy", None) or eng.copy
                cp_is.append(fn(out_sb[:, g, sl], pb[2 * g + h]))

            def out_store(mt, eng):
                dma_is.append(eng.dma_start(
                    out_d[mt * PT:(mt + 1) * PT, :], out_sb[:, mt, :]))

            # Interleaved PE order: s0 s1 g0 s2 s3 g1 g2 g3 - finished mm2
            # groups appear early so copies and stores spread out.
            # Copy halves (engine, in each queue's time order):
            #   Pool: at0:h0,h1  g0:h0,h1  at3:h1  g1:h0,h1  g2:h0  g3:h1
            #   DVE:  at1:h0,h1  at3:h0  g3:h0
            #   Act:  scratch  at2:h0,h1  g2:h1
            # Stores: g0,g3 on SP; g1,g2 on Act.
            pool, dve, act = nc.gpsimd, nc.vector, nc.scalar
            AT_ENG = {(0, 0): pool, (0, 1): pool,
                      (1, 0): dve, (1, 1): dve,
                      (2, 0): act, (2, 1): act,
                      (3, 0): dve, (3, 1): pool}
            OUT_ENG = {(0, 0): pool, (0, 1): pool,
                       (1, 0): pool, (1, 1): pool,
                       (2, 0): pool, (2, 1): act,
                       (3, 0): dve, (3, 1): pool}

            def sweep(kt):
                for mt in range(NT):
                    mm1_emit(kt, mt)
                    if mt % 2 == 1:
                        at_half(kt, mt // 2, AT_ENG[(kt, mt // 2)])

            def group(g):
                for k2 in range(NT):
                    mm2_emit(g, k2)
                    if k2 % 2 == 1:
                        out_half(g, k2 // 2, OUT_ENG[(g, k2 // 2)])

            sweep(0)
            sweep(1)
            group(0)
            sweep(2)
            sweep(3)
            out_store(0, nc.sync)
            group(1)
            out_store(1, nc.scalar)
            group(2)
            group(3)
            out_store(2, nc.scalar)
            out_store(3, nc.sync)

            # Pre-observe every DMA on its OWN engine (queue order makes the
            # wait instant - a cross-engine observer would park on the DMA
            # sem and eat the ~1.7us DMA-completion latency), and each engine
            # group on SP, so the auto-generated kernel-tail Drain and final
            # barrier carry no waits of their own.
            eng_of = {mybir.EngineType.SP: nc.sync,
                      mybir.EngineType.Activation: nc.scalar,
                      mybir.EngineType.Pool: nc.gpsimd,
                      mybir.EngineType.DVE: nc.vector,
                      mybir.EngineType.PE: nc.tensor}
            for d in dma_is:
                n = eng_of[d.ins.engine].nop(nofuse=True)
                tile.add_dep_helper(n.ins, d.ins, True, "pre-drain observe")
            for group in (mm_is, cp_is, [warm_i]):
                n = nc.sync.nop(nofuse=True)
                for d in group:
                    tile.add_dep_helper(n.ins, d.ins, True, "pre-drain observe")

    return nc


def _in_maps(input_state: np.ndarray, angles: np.ndarray) -> list[dict]:
    import ml_dtypes

    u = _build_u(np.asarray(angles, np.float64))
    bt = u[_PERM][:, _PERM].T.astype(np.float32)  # B^T, block-diagonal
    rho = np.asarray(input_state, np.float32)[:, _PERM][:, :, _PERM]
    out = []
    for b in range(N_CORES):
        inp = np.empty((DIM, ROW), ml_dtypes.bfloat16)
        inp[:, :DIM] = rho[b]
        for kt in range(NT):
            band = slice(kt * PT, (kt + 1) * PT)
            inp[band, DIM:] = bt[band, band]
        out.append({"inp": inp})
    return out


def kernel(input_state: np.ndarray, angles: np.ndarray) -> np.ndarray:
    from concourse.bass_utils import run_bass_kernel_spmd

    if "nc" not in _NC_CACHE:
        _NC_CACHE["nc"] = _build_bass()
    nc = _NC_CACHE["nc"]

    in_maps = _in_maps(input_state, angles)
    res = run_bass_kernel_spmd(nc, in_maps, core_ids=list(range(N_CORES)))
    out = np.stack([np.asarray(res.results[b]["out"], np.float32)
                    for b in range(N_CORES)], axis=0)
    out = np.ascontiguousarray(out[:, _INV_PERM][:, :, _INV_PERM])
    return out.astype(np.float32)


# revision 25
# speedup vs baseline: 1.0708x; 1.0708x over previous
"""Trainium2 kernel for nn_Dense_RBS_density: rho <- U rho U^T over a batch
of 8 density matrices in the Hamming-weight-2 basis of 32 qubits (dim=496).

The 15 RBS gates act on disjoint qubit pairs, so they commute and fold into a
single orthogonal matrix U (built on host from the 15 angles — negligible
work). In a permuted basis U is block-diagonal with four 124x124 blocks
(each itself made of <=4x4 rotations), so per core (one batch element):
    mm1 bank kt: A^T[mt,kt] = rho'[kt,mt]^T @ B^T[kt,kt]   (16 matmuls)
    mm2 bank mt: out'[mt,kt] = A[mt,kt] @ B^T[kt,kt]       (16 matmuls)

Everything is bf16 (inputs, matmuls, PSUM results, stores): the harness
tolerance (2e-2) dwarfs bf16 rounding (~5e-3), bf16 matmuls stream 1 row per
PE cycle at any p-state (f32r pays 2-4x for 124-wide outputs), and bf16
halves DMA bytes.

Schedule: all four mm1 k-sweeps run back-to-back on the PE while DVE chases
them with PSUM->SBUF at-copies; the four mm2 groups follow; each finished
ps2 bank is DMA'd straight from PSUM to HBM (no SBUF staging, no Activation
copies -> no one-time 1.3us ACT table load). Input is loaded as 4 chunks on
4 different engine DMA queues so the whole load fits in one ~500ns slot.
"""

import itertools
import math

import numpy as np

N_QUBITS = 32
LIST_GATES = [(2 * i, 2 * i + 1) for i in range(15)]
DIM = 496  # C(32, 2)
PT = 124  # partition tile size; 4 * 124 = 496
NT = 4  # number of tiles along each axis
N_CORES = 8
N_WARMUP_MM = 1  # dummy matmul that delays the PE past the chunk-0 release
ROW = DIM + PT  # packed input row: 496 rho' columns + 124 block columns


def _gate_pairs():
    """For each gate (a,b), the list of (k, kp) basis-index pairs rotated by
    the gate: k contains a, kp = same state with a replaced by b."""
    pairs = list(itertools.combinations(range(N_QUBITS), 2))
    idx = {p: k for k, p in enumerate(pairs)}
    out = []
    for a, b in LIST_GATES:
        rot = []
        for p, k in idx.items():
            if (a in p) and (b not in p):
                other = p[0] if p[1] == a else p[1]
                kp = idx[tuple(sorted((other, b)))]
                rot.append((k, kp))
        out.append(rot)
    return out


_GATE_PAIRS = _gate_pairs()


def _build_perm():
    """Basis order that block-diagonalizes the folded U: 105 4-blocks (gate
    pair x gate pair), 30 2-blocks (qubit 30/31 partners), 16 fixed states.
    All blocks land inside aligned 124-wide tiles."""
    pairs = list(itertools.combinations(range(N_QUBITS), 2))
    idx = {p: k for k, p in enumerate(pairs)}
    perm = []
    for a in range(15):
        for b in range(a + 1, 15):
            for x in range(2):
                for y in range(2):
                    perm.append(idx[(2 * a + x, 2 * b + y)])
    for q in (30, 31):
        for a in range(15):
            perm.append(idx[tuple(sorted((2 * a, q)))])
            perm.append(idx[tuple(sorted((2 * a + 1, q)))])
    for a in range(15):
        perm.append(idx[(2 * a, 2 * a + 1)])
    perm.append(idx[(30, 31)])
    return np.array(perm)


_PERM = _build_perm()
_INV_PERM = np.argsort(_PERM)


def _build_u(angles: np.ndarray) -> np.ndarray:
    """Fold the 15 commuting RBS gates into one orthogonal DIMxDIM matrix."""
    u = np.eye(DIM, dtype=np.float64)
    for g, rot in enumerate(_GATE_PAIRS):
        c = math.cos(float(angles[g]))
        s = math.sin(float(angles[g]))
        k = np.array([r[0] for r in rot])
        kp = np.array([r[1] for r in rot])
        rk, rkp = u[k].copy(), u[kp].copy()
        u[k] = c * rk + s * rkp
        u[kp] = -s * rk + c * rkp
    return u


_NC_CACHE = {}


def _build_bass():
    import concourse.bass as bass
    import concourse.mybir as mybir
    import concourse.tile as tile
    from concourse.bass import MemorySpace

    mm_dt = mybir.dt.bfloat16

    nc = bass.Bass("TRN2", target_bir_lowering=False, debug=False)
    # 4 chunks of [rho k-tile; U^T k-tile], 124 rows each, bf16.
    inp_d = nc.dram_tensor("inp", [DIM, ROW], mm_dt,
                           kind="ExternalInput").ap()
    out_d = nc.dram_tensor("out", [DIM, DIM], mm_dt,
                           kind="ExternalOutput").ap()

    with tile.TileContext(nc) as tc:
        with (
            tc.tile_pool(name="consts", bufs=1) as consts,
            tc.tile_pool(name="psum", bufs=1, space=MemorySpace.PSUM) as psum,
        ):
            # [124, 4, 620]: per k-tile, 496 rho' columns + this tile's
            # 124x124 diagonal block of B^T.
            inp_sb = consts.tile([PT, NT, ROW], mm_dt, tag="inp")
            at_sb = consts.tile([PT, NT, DIM], mm_dt, tag="at")
            out_sb = consts.tile([PT, NT, DIM], mm_dt, tag="outs")
            warm_sb = consts.tile([PT, 280], mm_dt, tag="warm")

            dma_is, mm_is, cp_is = [], [], []
            # In this simulator a consumer that parks on a DMA semaphore
            # before the DMA's engine-release pays the full ~1.7us DMA init
            # latency; one that checks after release proceeds immediately.
            # So every DMA-sem consumer must ARRIVE LATE: the PE runs one
            # small warmup matmul (gated on a DVE memset sized so the PE
            # reaches the chunk-0 wait just after the load's 500ns engine
            # slot has drained).
            warm_i = nc.vector.memset(warm_sb, 0.0)
            # input chunks: c0/c2/c3 on SP, c1 on Act (both first-needed
            # chunks get first queue slots; Pool gets NO DMA - SWDGE sems
            # resolve ~2.4us late and Pool must be free for copies anyway)
            engs = [nc.sync, nc.scalar, nc.sync, nc.sync]
            for kt in range(NT):
                dma_is.append(engs[kt].dma_start(
                    inp_sb[:, kt, :],
                    inp_d[PT * kt:PT * (kt + 1), :]))
            # Activation's one-time table load (1.4us), paid while the PE
            # does mm1 - Act's first real copy comes much later
            scratch_sb = consts.tile([PT, 1], mybir.dt.float32, tag="scratch")
            cp_is.append(nc.scalar.copy(scratch_sb, warm_sb[:, :1]))

            # 8 PSUM tiles of two 124-slices each. Tile j=2k+m//2 holds mm1
            # slices (k, 2(j%2)) and (k, 2(j%2)+1); mm2 group g reuses tile
            # 2g+k2//2 long after its mm1 readers finished. Each tile gets
            # BOTH its writes before its single half-copy read, so a matmul
            # never stalls on an in-flight PSUM read (tile-level WAR).
            pb = [psum.tile([PT, 2 * PT], mybir.dt.float32, tag=f"pb{j}",
                            name=f"pb{j}") for j in range(2 * NT)]
            for _ in range(N_WARMUP_MM):
                mm_is.append(nc.tensor.matmul(
                    pb[0][:, :PT], warm_sb[:, :PT], warm_sb[:, :PT],
                    start=True, stop=True))

            def mm1_emit(kt, mt):
                mm_is.append(nc.tensor.matmul(
                    pb[2 * kt + mt // 2][:, (mt % 2) * PT:(mt % 2 + 1) * PT],
                    inp_sb[:, kt, mt * PT:(mt + 1) * PT],
                    inp_sb[:, kt, DIM:ROW],
                    start=True, stop=True,
                ))

            def mm2_emit(g, k2):
                # needs region (m-band k2, n-band g) = at tile g, slice k2
                mm_is.append(nc.tensor.matmul(
                    pb[2 * g + k2 // 2][:, (k2 % 2) * PT:(k2 % 2 + 1) * PT],
                    at_sb[:, g, k2 * PT:(k2 + 1) * PT],
                    inp_sb[:, k2, DIM:ROW],
                    start=True, stop=True,
                ))

            def at_half(kt, h, eng):
                sl = slice(h * 2 * PT, (h + 1) * 2 * PT)
                fn = getattr(eng, "tensor_copy", None) or eng.copy
                cp_is.append(fn(at_sb[:, kt, sl], pb[2 * kt + h]))

            def out_half(g, h, eng):
                sl = slice(h * 2 * PT, (h + 1) * 2 * PT)
                fn = getattr(eng, "tensor_copy", None) or eng.copy
                cp_is.append(fn(out_sb[:, g, sl], pb[2 * g + h]))

            def out_store(mt, eng):
                dma_is.append(eng.dma_start(
                    out_d[mt * PT:(mt + 1) * PT, :], out_sb[:, mt, :]))

            # Interleaved PE order: s0 s1 g0 s2 s3 g1 g2 g3 - finished mm2
            # groups appear early so copies and stores spread out.
            # Copy halves (engine, in each queue's time order):
            #   Pool: at0:h0,h1  g0:h0,h1  at3:h1  g1:h0,h1  g2:h0  g3:h1
            #   DVE:  at1:h0,h1  at3:h0  g3:h0
            #   Act:  scratch  at2:h0,h1  g2:h1
            # Stores: g0,g1,g3 on SP; g2 on Act (after its g2:h1 copy).
            pool, dve, act = nc.gpsimd, nc.vector, nc.scalar
            AT_ENG = {(0, 0): pool, (0, 1): pool,
                      (1, 0): dve, (1, 1): dve,
                      (2, 0): act, (2, 1): act,
                      (3, 0): dve, (3, 1): pool}
            OUT_ENG = {(0, 0): pool, (0, 1): pool,
                       (1, 0): pool, (1, 1): pool,
                       (2, 0): pool, (2, 1): act,
                       (3, 0): dve, (3, 1): pool}

            def sweep(kt):
                for mt in range(NT):
                    mm1_emit(kt, mt)
                    if mt % 2 == 1:
                        at_half(kt, mt // 2, AT_ENG[(kt, mt // 2)])

            def group(g):
                for k2 in range(NT):
                    mm2_emit(g, k2)
                    if k2 % 2 == 1:
                        out_half(g, k2 // 2, OUT_ENG[(g, k2 // 2)])

            sweep(0)
            sweep(1)
            group(0)
            sweep(2)
            sweep(3)
            out_store(0, nc.sync)
            group(1)
            out_store(1, nc.sync)
            group(2)
            group(3)
            out_store(2, nc.scalar)
            out_store(3, nc.sync)

            # Pre-observe every DMA on its OWN engine (queue order makes the
            # wait instant - a cross-engine observer would park on the DMA
            # sem and eat the ~1.7us DMA-completion latency), and each engine
            # group on SP, so the auto-generated kernel-tail Drain and final
            # barrier carry no waits of their own.
            eng_of = {mybir.EngineType.SP: nc.sync,
                      mybir.EngineType.Activation: nc.scalar,
                      mybir.EngineType.Pool: nc.gpsimd,
                      mybir.EngineType.DVE: nc.vector,
                      mybir.EngineType.PE: nc.tensor}
            for d in dma_is:
                n = eng_of[d.ins.engine].nop(nofuse=True)
                tile.add_dep_helper(n.ins, d.ins, True, "pre-drain observe")
            for group in (mm_is, cp_is, [warm_i]):
                n = nc.sync.nop(nofuse=True)
                for d in group:
                    tile.add_dep_helper(n.ins, d.ins, True, "pre-drain observe")

    return nc


def _in_maps(input_state: np.ndarray, angles: np.ndarray) -> list[dict]:
    import ml_dtypes

    u = _build_u(np.asarray(angles, np.float64))
    bt = u[_PERM][:, _PERM].T.astype(np.float32)  # B^T, block-diagonal
    rho = np.asarray(input_state, np.float32)[:, _PERM][:, :, _PERM]
    out = []
    for b in range(N_CORES):
        inp = np.empty((DIM, ROW), ml_dtypes.bfloat16)
        inp[:, :DIM] = rho[b]
        for kt in range(NT):
            band = slice(kt * PT, (kt + 1) * PT)
            inp[band, DIM:] = bt[band, band]
        out.append({"inp": inp})
    return out


def kernel(input_state: np.ndarray, angles: np.ndarray) -> np.ndarray:
    from concourse.bass_utils import run_bass_kernel_spmd

    if "nc" not in _NC_CACHE:
        _NC_CACHE["nc"] = _build_bass()
    nc = _NC_CACHE["nc"]

    in_maps = _in_maps(input_state, angles)
    res = run_bass_kernel_spmd(nc, in_maps, core_ids=list(range(N_CORES)))
    out = np.stack([np.asarray(res.results[b]["out"], np.float32)
                    for b in range(N_CORES)], axis=0)
    out = np.ascontiguousarray(out[:, _INV_PERM][:, :, _INV_PERM])
    return out.astype(np.float32)
